# revision 36
# baseline (speedup 1.0000x reference)
"""Multi-head attention block (B=8, S=1024, D=1024, H=16) on 8 TRN2 NeuronCores.

Data-parallel over batch: core i computes batch element i end-to-end.
Per-core dataflow (bf16 compute, f32 PSUM accumulation; x/W pre-cast to
bf16 on the host):
  xT = transpose(x)  (host)
  qkT[n,s] = W_qkv[:, :2048]^T @ x^T     (q rows pre-scaled by hd^-0.5)
  v[s,n]   = x @ W_qkv[:, 2048:]         (head-interleaved with a ones
                                          column per head -> "va", M=65)
  per head-pair hp, key-tile jt:
    scoresT[kj,qi] = kT^T q   as ROW-TILED CONCURRENT pairs: head-even on
    PE rows 0-63 (tile_position (0,0)), head-odd on rows 64-127 ((64,0)).
    Adjacent disjoint-row-group matmuls execute concurrently on TRN2
    (measured 161 ns/MM vs 470 serial), halving scores PE time.
    expT = exp(scoresT)                  (ScalarE, PSUM -> SBUF bf16)
  outT[c,qi], Z[qi] = va^T @ expT        (ones column accumulates Z)
  outT = outT * (1/Z)                    (recip + partition bcast, fused
                                          normalize-copy out of PSUM)
  out = outT^T @ W_proj (+ corr on host)

The schedule is a single software-pipelined loop over hp with per-jt slot
interleaving so the PE queue never head-of-line blocks on ScalarE exp:
each slot emits [4 scores MMs][filler chunk], where filler rotates through
v-phase (iter 0), attnv(hp-1) groups, qkv(hp+2) halves, and proj partial
chunks (iter 7). Proj accumulates kt 0..5 into PSUM during iter 7, dumps
partials to SBUF, and finishes kt 6..7 + add in the tail.
"""

import sys

if "/opt/trn_rl_repo" not in sys.path:
    sys.path.insert(0, "/opt/trn_rl_repo")

import ml_dtypes
import numpy as np

P = 128
S = 1024
D = 1024
H = 16
HD = 64
N_CORES = 8
SCALE = HD ** -0.5
ST = S // P   # 8 s-tiles
DT = D // P   # 8 d-tiles (contraction tiles)

_CACHE = {}


def _build():
    if "nc" in _CACHE:
        return _CACHE["nc"]

    from contextlib import ExitStack

    import concourse.bass as bass  # noqa: F401
    import concourse.mybir as mybir
    import concourse.tile as tile
    from concourse import bacc
    F32 = mybir.dt.float32
    BF = mybir.dt.bfloat16
    AluOp = mybir.AluOpType
    Act = mybir.ActivationFunctionType

    nc = bacc.Bacc(
        "TRN2", target_bir_lowering=False, debug=False, num_devices=N_CORES
    )

    x_d = nc.dram_tensor("x", [D, S], BF, kind="ExternalInput")  # x^T
    wqkv_d = nc.dram_tensor("W_qkv", [D, 3 * D], BF, kind="ExternalInput")
    bqkv_d = nc.dram_tensor("b_qkv", [3 * D], F32, kind="ExternalInput")
    wproj_d = nc.dram_tensor("W_proj", [D, D], BF, kind="ExternalInput")
    bproj_d = nc.dram_tensor("b_proj", [D], F32, kind="ExternalInput")
    # bf16 output: host upcasts; halves the out-DMA drain at the tail
    out_d = nc.dram_tensor("out", [S, D], BF, kind="ExternalOutput")

    with tile.TileContext(nc) as tc, ExitStack() as ctx:
        const = ctx.enter_context(tc.tile_pool(name="const", bufs=1))
        persist = ctx.enter_context(tc.tile_pool(name="persist", bufs=1))
        # PSUM budget (8 banks of [128,2KB]):
        #   sc: scores psum [128,1024] x3 rotating (6 banks) — the 3-way
        #       rotation staggers the exp-release gates so a row-paired
        #       A/B slot never waits on the *adjacent* exp
        #   sm: qkv / v / attnv / proj chunks [128,512] x2 (2 banks)
        ps_sc = ctx.enter_context(tc.tile_pool(name="sc", bufs=3, space="PSUM"))
        ps_sm = ctx.enter_context(tc.tile_pool(name="sm", bufs=2, space="PSUM"))
        small = ctx.enter_context(tc.tile_pool(name="small", bufs=2))

        # ---- constants ----
        zbias = const.tile([P, 1], F32)  # zero bias for activation(Exp)
        nc.gpsimd.memset(zbias[:], 0.0)

        # b_qkv q,k part: host passes it permuted to [p, nt] layout -> one DMA
        bqcol = const.tile([P, 16], F32)

        # ---- persistent tensors ----
        qk_pool = ctx.enter_context(tc.tile_pool(name="qk", bufs=6))
        va = [persist.tile([P, H * (HD + 1)], BF, name=f"va{s}") for s in range(ST)]
        outT = [persist.tile([P, S], BF, name=f"outT{t}") for t in range(DT)]
        # merged per-d-tile tensors: few wide DMA descriptors instead of
        # dozens of per-tile ones (the Sync engine issues descriptors at
        # ~0.6-0.9us each — 57 descriptors serialized ~20us of startup)
        xall = persist.tile([P, DT * S], BF, name="xall")
        WqkE = persist.tile([P, DT * 4 * P], BF, name="WqkE")
        WQK_W = 2 * D - 4 * P  # 1536 q+k columns per d-tile
        Wqk = persist.tile([P, DT * WQK_W], BF, name="Wqk", tag="wsh")
        # Wp reuses the Wqk slot (dead after the last qkv pair) via shared tag
        Wp = persist.tile([P, DT * D], BF, name="Wp", tag="wsh")
        # proj partial accumulators (kt 0..5), bf16
        prt = [persist.tile([P, D], BF, name=f"prt{t}") for t in range(ST)]

        for s8 in range(ST):
            nc.gpsimd.memset(va[s8][:], 1.0)  # ones columns survive the v copies

        # ---- DMA loads (bf16, pre-cast + pre-transposed x on host) ----
        # WqkE first (small), then x; chunked into a few descriptors each
        # (descriptor issue is serial on Sync; chunks keep queue parallelism)
        wsrc = wqkv_d.rearrange("(t p) c -> p t c", p=P)
        wqke3 = WqkE[:].rearrange("p (t c) -> p t c", c=4 * P)
        nc.sync.dma_start(wqke3[:, :, 0 : 2 * P], wsrc[:, :, 0 : 2 * P])
        nc.sync.dma_start(wqke3[:, :, 2 * P : 4 * P], wsrc[:, :, D : D + 2 * P])
        nc.sync.dma_start(bqcol[:], bqkv_d[: 2 * D].rearrange("(p t) -> p t", t=16))
        xsrc = x_d.rearrange("(t p) s -> p t s", p=P)
        xall3 = xall[:].rearrange("p (t s) -> p t s", s=S)
        for c in range(4):
            nc.sync.dma_start(
                xall3[:, 2 * c : 2 * c + 2, :], xsrc[:, 2 * c : 2 * c + 2, :]
            )

        qkT = {}

        def qkv_half(hp, which):
            """One qkv output for pair hp: which=0 -> q (scaled), 1 -> k.
            16 MMs into the big psum + 1 DVE tensor_scalar."""
            nt = hp if which == 0 else 8 + hp
            dst = qk_pool.tile([P, S], BF, name=f"qk{nt}", tag="qk")
            if hp not in qkT:
                qkT[hp] = [None, None]
            qkT[hp][which] = dst
            flush_av_mul()  # release the neighbor attnv group's pso early
            for sh in range(2):
                ps = ps_sm.tile([P, 512], F32, name="ps_qk", tag="sm")
                for dt2 in range(DT):
                    if hp < 2:
                        c0 = dt2 * 4 * P + (which * 2 + hp) * P
                        w_ap = WqkE[:, c0 : c0 + P]
                    else:
                        c0 = dt2 * WQK_W + ((hp - 2) + which * 6) * P
                        w_ap = Wqk[:, c0 : c0 + P]
                    nc.tensor.matmul(
                        ps[:],
                        w_ap,
                        xall[:, dt2 * S + sh * 512 : dt2 * S + (sh + 1) * 512],
                        start=(dt2 == 0),
                        stop=(dt2 == DT - 1),
                    )
                dsth = dst[:, sh * 512 : (sh + 1) * 512]
                if which == 0:  # q: (psum + b) * scale
                    nc.vector.tensor_scalar(
                        dsth, ps[:], bqcol[:, nt : nt + 1], SCALE,
                        AluOp.add, AluOp.mult,
                    )
                else:  # k: psum + b
                    nc.vector.tensor_scalar_add(dsth, ps[:], bqcol[:, nt : nt + 1])

        def v_chunk(Wv, s8, sh):
            """Half v-phase block: 8 MMs into an sm psum + 1 DVE copy into va."""
            ps = ps_sm.tile([P, 512], F32, name="ps_v", tag="sm")
            for dt2 in range(DT):
                nc.tensor.matmul(
                    ps[:],
                    xall[:, dt2 * S + s8 * P : dt2 * S + (s8 + 1) * P],
                    Wv[:, dt2 * D + sh * 512 : dt2 * D + (sh + 1) * 512],
                    start=(dt2 == 0),
                    stop=(dt2 == DT - 1),
                )
            nc.vector.tensor_copy(
                va[s8][:].rearrange("p (h c) -> p h c", c=HD + 1)[
                    :, sh * 8 : (sh + 1) * 8, 0:HD
                ],
                ps[:].rearrange("p (h c) -> p h c", c=HD),
            )

        exp_tiles = {}

        def scores_slot(hp, jt, exp_pool):
            """Row-tiled concurrent scores for pair hp, key-tile jt:
            A (head even, PE rows 0-63) and B (head odd, rows 64-127)."""
            if hp not in exp_tiles:
                expA = exp_pool.tile([P, ST * S], BF, name="expA", tag="expA")
                expB = exp_pool.tile([P, ST * S], BF, name="expB", tag="expB")
                exp_tiles[hp] = (expA, expB)
            expA, expB = exp_tiles[hp]
            qtile, ktile = qkT[hp][0], qkT[hp][1]
            psA = ps_sc.tile([P, S], F32, name="psA", tag="sc")
            psB = ps_sc.tile([P, S], F32, name="psB", tag="sc")
            for sh in range(2):
                nc.tensor.matmul(
                    psA[:, sh * 512 : (sh + 1) * 512],
                    ktile[0:64, jt * P : (jt + 1) * P],
                    qtile[0:64, sh * 512 : (sh + 1) * 512],
                    tile_position=(0, 0),
                )
                nc.tensor.matmul(
                    psB[:, sh * 512 : (sh + 1) * 512],
                    ktile[64:128, jt * P : (jt + 1) * P],
                    qtile[64:128, sh * 512 : (sh + 1) * 512],
                    tile_position=(64, 0),
                )
            nc.scalar.activation(
                expA[:, jt * S : (jt + 1) * S], psA[:], Act.Exp, bias=zbias[:]
            )
            nc.scalar.activation(
                expB[:, jt * S : (jt + 1) * S], psB[:], Act.Exp, bias=zbias[:]
            )

        pending_mul = []

        def flush_av_mul():
            # deferred normalize-copy: emitted late so the DVE ops queued
            # between attnv groups (pp dumps, fin adds) are not head-of-line
            # blocked behind the cross-engine wait on the gpsimd broadcast
            while pending_mul:
                reg, pso, bz, po = pending_mul.pop()
                nc.vector.tensor_mul(reg, pso[0:64, :], bz[po : po + 64, :])

        def attnv_group(hp, g):
            """One attnv accumulation group: g in 0..3 -> (head, qh).
            8 MMs into an sm psum [65,512] + normalize chain."""
            flush_av_mul()
            head = 2 * hp + (g // 2)
            qh = g % 2
            ex = exp_tiles[hp][g // 2]
            pso = ps_sm.tile([P, 512], F32, name="pso", tag="sm")
            for jt in range(ST):
                nc.tensor.matmul(
                    pso[0:65, :],
                    va[jt][:, head * 65 : head * 65 + 65],
                    ex[:, jt * S + qh * 512 : jt * S + qh * 512 + 512],
                    start=(jt == 0),
                    stop=(jt == ST - 1),
                )
            po = (head % 2) * 64
            reg = outT[hp][po : po + 64, qh * 512 : (qh + 1) * 512]
            zs = small.tile([1, 512], F32, name="zs", tag="zs")
            nc.vector.tensor_copy(zs[:], pso[64:65, :])
            rz = small.tile([1, 512], F32, name="rz", tag="rz")
            nc.vector.reciprocal_approx_fast(out=rz[:], in_=zs[:])
            bz = small.tile([P, 512], F32, name="bz", tag="bz")
            nc.gpsimd.partition_broadcast(bz[:], rz[:])
            pending_mul.append((reg, pso, bz, po))

        def attnv_cleanup(hp):
            exp_tiles.pop(hp, None)

        ob_pool = ctx.enter_context(tc.tile_pool(name="obp", bufs=2))

        def proj_partial(st, sh, nkt):
            """Accumulate kt 0..nkt-1 for (st, sh), dump to SBUF partial."""
            flush_av_mul()  # release the neighbor attnv group's pso early
            ps = ps_sm.tile([P, 512], F32, name="ps_pp", tag="sm")
            for kt in range(nkt):
                nc.tensor.matmul(
                    ps[:],
                    outT[kt][:, st * P : (st + 1) * P],
                    Wp[:, kt * D + sh * 512 : kt * D + (sh + 1) * 512],
                    start=(kt == 0),
                    stop=(kt == nkt - 1),
                )
            nc.vector.tensor_copy(prt[st][:, sh * 512 : (sh + 1) * 512], ps[:])

        def proj_final(st, kt0):
            """kt kt0..7 for both sh halves in one sc tile + add partial + DMA.
            Runs in the tail where the sc pool is free after the last exps."""
            ps = ps_sc.tile([P, S], F32, name="ps_pf", tag="sc")
            for sh in range(2):
                for kt in range(kt0, DT):
                    nc.tensor.matmul(
                        ps[:, sh * 512 : (sh + 1) * 512],
                        outT[kt][:, st * P : (st + 1) * P],
                        Wp[:, kt * D + sh * 512 : kt * D + (sh + 1) * 512],
                        start=(kt == kt0),
                        stop=(kt == DT - 1),
                    )
            ob = ob_pool.tile([P, S], BF, name="ob", tag="ob")
            nc.vector.tensor_tensor(ob[:], ps[:], prt[st][:], AluOp.add)
            nc.sync.dma_start(out_d[st * P : (st + 1) * P, :], ob[:])

        # ---- software-pipelined schedule ----
        with tc.tile_pool(name="xv", bufs=1) as xv_pool, \
             tc.tile_pool(name="exp", bufs=2) as exp_pool:
            Wv = xv_pool.tile([P, DT * D], BF, name="Wv")
            wv3 = Wv[:].rearrange("p (t c) -> p t c", c=D)
            for c in range(2):
                nc.sync.dma_start(
                    wv3[:, 4 * c : 4 * c + 4, :],
                    wsrc[:, 4 * c : 4 * c + 4, 2 * D : 3 * D],
                )
            wqk3 = Wqk[:].rearrange("p (t c) -> p t c", c=WQK_W)
            for c in range(2):
                nc.sync.dma_start(
                    wqk3[:, 4 * c : 4 * c + 4, 0 : D - 2 * P],
                    wsrc[:, 4 * c : 4 * c + 4, 2 * P : D],
                )
                nc.sync.dma_start(
                    wqk3[:, 4 * c : 4 * c + 4, D - 2 * P : WQK_W],
                    wsrc[:, 4 * c : 4 * c + 4, D + 2 * P : 2 * D],
                )
            qkv_half(0, 0)
            qkv_half(0, 1)
            qkv_half(1, 0)
            qkv_half(1, 1)

            for hp in range(8):
                if hp == 6:
                    # last qkv pair emitted in iter 5; Wp DMA waits on the
                    # Wqk slot free automatically (shared tag)
                    wp3 = Wp[:].rearrange("p (t c) -> p t c", c=D)
                    psrc = wproj_d.rearrange("(t p) c -> p t c", p=P)
                    for c in range(2):
                        nc.sync.dma_start(
                            wp3[:, 4 * c : 4 * c + 4, :],
                            psrc[:, 4 * c : 4 * c + 4, :],
                        )
                # build this iteration's filler chunk list
                fillers = []
                if hp == 0:
                    # v-phase: 16 chunks; qkv(2): 2 chunks
                    for s8 in range(ST):
                        for sh in range(2):
                            fillers.append(("v", s8, sh))
                    fillers.insert(3, ("qkv", 2, 0))
                    fillers.insert(9, ("qkv", 2, 1))
                elif hp <= 5:
                    for g in range(4):
                        fillers.append(("av", hp - 1, g))
                    fillers.insert(1, ("qkv", hp + 2, 0))
                    fillers.insert(3, ("qkv", hp + 2, 1))
                else:
                    # iters 6/7: attnv(hp-1) groups interleaved among proj
                    # partials (st 0..3 with kt<=4 in iter 6 — outT[5] is not
                    # complete until av(5)'s last group — st 4..7 with kt<=5
                    # in iter 7) so normalize chains don't serialize the sm pool
                    st0, nkt = (0, 5) if hp == 6 else (4, 6)
                    pp = [
                        ("pp", st, sh, nkt)
                        for st in range(st0, st0 + 4)
                        for sh in range(2)
                    ]
                    for g in range(4):
                        fillers.append(("av", hp - 1, g))
                        fillers.extend(pp[2 * g : 2 * g + 2])
                # round-robin: one scores slot, then fillers spread across slots
                fi = 0
                for jt in range(ST):
                    scores_slot(hp, jt, exp_pool)
                    # emit fillers paced across the 8 jt slots
                    want = (jt + 1) * len(fillers) // ST
                    while fi < want:
                        kind, *args = fillers[fi]
                        if kind == "v":
                            v_chunk(Wv, *args)
                        elif kind == "qkv":
                            qkv_half(*args)
                        elif kind == "av":
                            attnv_group(*args)
                        elif kind == "pp":
                            proj_partial(*args)
                        fi += 1
                if hp >= 1:
                    attnv_cleanup(hp - 1)

            # tail: attnv(7) trails the last exps; proj finals interleave.
            # fin(st<4) reads outT[7] rows written by groups 0 (qh0 even) and
            # 2 (qh0 odd); fin(st>=4) needs groups 1 and 3.
            attnv_group(7, 0)
            attnv_group(7, 2)
            flush_av_mul()
            for st in range(4):
                proj_final(st, kt0=5)
            attnv_group(7, 1)
            attnv_group(7, 3)
            flush_av_mul()
            for st in range(4, ST):
                proj_final(st, kt0=6)
            attnv_cleanup(7)

    nc.compile()
    _CACHE["nc"] = nc
    return nc


def kernel(x, W_qkv, b_qkv, W_proj, b_proj, _trace=False):
    nc = _build()
    from concourse.bass_utils import run_bass_kernel_spmd

    bf = ml_dtypes.bfloat16
    wq = np.ascontiguousarray(W_qkv, dtype=np.float32).astype(bf)
    wp = np.ascontiguousarray(W_proj, dtype=np.float32).astype(bf)
    bq0 = np.asarray(b_qkv, dtype=np.float32)
    bq = np.concatenate(
        [np.ascontiguousarray(bq0[:2048].reshape(16, 128).T).ravel(), bq0[2048:]]
    ).astype(np.float32)
    bp = np.ascontiguousarray(b_proj, dtype=np.float32)
    in_maps = []
    for i in range(N_CORES):
        in_maps.append(
            {
                "x": np.ascontiguousarray(np.asarray(x[i], dtype=np.float32).T).astype(bf),
                "W_qkv": wq,
                "b_qkv": bq,
                "W_proj": wp,
                "b_proj": bp,
            }
        )
    res = run_bass_kernel_spmd(
        nc, in_maps, core_ids=list(range(N_CORES)), trace=_trace
    )
    out = np.stack(
        [np.asarray(res.results[i]["out"]) for i in range(N_CORES)], axis=0
    ).astype(np.float32)
    # v-bias and proj-bias applied exactly on the host:
    # out = (attn + 1*bv) @ Wp + bp  ==  attn @ Wp  +  (bv @ Wp + bp)
    corr = np.asarray(b_qkv, np.float32)[2 * D :] @ np.asarray(W_proj, np.float32)
    corr = corr + np.asarray(b_proj, np.float32)
    if np.any(corr):
        out += corr[None, None, :]
    if _trace:
        _CACHE["last_results"] = res
    return out
